# revision 88
# baseline (speedup 1.0000x reference)
"""DCGRU cell (DCRNN) Trainium2 Bass kernel — fp8 diffusion version.

Strategy: data-parallel over batch B=64 across 8 NeuronCores (8 batches per
core); supports + gconv weights replicated.

Math restructuring (validated in numpy against the jax reference):
  reference diffusion xs = [x0, S0@x0, 2*S0^2@x0 - x0, S1@S0@x0, 2*S1^2@S0@x0 - S0@x0]
  -> raw chain     ys = [y0, y1=S0@y0, y2=S0@y1, y3=S1@y1, y4=S1@y3]
  with the 2a-b combinations folded into the projection weights on the host:
  What = [W0-W2, W1-W4, 2*W2, W3, 2*W4] (Wm = rows insz*5+m of the gconv W).

fp8 acceleration: the diffusion hops run as float8e4 DoubleRow matmuls
(double-pumped PE: 2 fp8 rows/cycle, 256-deep contraction per instruction;
measured ~4x fp32r throughput on HW) with 4x less support HBM traffic.
Supports are host-quantized to e4m3 scaled by 2^17 (entries in [0.06, 71] —
all normal range); X-states are quantized on-device with per-hop power-of-2
scales (x0: 2^5, hops: 2^8) applied during the PSUM->SBUF requant copy (ACT).
Hop-term quantization is benign because the projection is dominated by its
m=0 term (hop outputs are ~50x smaller).  Measured end-to-end rel err 3.3e-3
vs the fp32 reference (harness gate 2e-2).

Per-core device layout:
  Diffusion state X [N, C=528] fp8 in SBUF, columns c = b*64+u (hx part) then
  512 + b*2 + j (input part); bufA/bufB are shared by both gconvs (p0
  rebuilds gconv2's x0 into bufA in place via the xs4 un-transpose, no DRAM
  round trip).  S0 is streamed once per gconv into a resident SBUF tile
  (hop 2 reuses it with no DMA); S1 is streamed per hop in 2-block chunks.
  After each hop the fp8 state is PE-transposed (step-2 PSUM out, paired
  tiles) and spilled to DRAM as YT [640, N] fp8 in 4-block groups (512B HBM
  runs).  Projection: ZT_b[out,n] = sum_m What_m.T @ YT_m[b-rows, n]
  accumulated tau-scaled (2^12) in one PSUM group: m=0 bf16 x bf16, m=1..4
  as two DoubleRow fp8 pairs (weights hold What*TAU/AH); the activation
  (sigmoid/tanh) descales via its scale operand and adds the bias.  Gate
  tensors (hx, u, r*hx) travel as bf16; gate arithmetic is split across
  DVE/Pool; group outputs are batched into single DMAs.  Host un-transposes
  the final output during unsharding.
"""

import os
from contextlib import ExitStack

import numpy as np
import ml_dtypes

import concourse.bacc as bacc
import concourse.mybir as mybir
import concourse.tile as tile
from concourse.bass_utils import run_bass_kernel_spmd
from concourse.masks import make_identity

F32 = mybir.dt.float32
F32R = mybir.dt.float32r
F8 = mybir.dt.float8e4
BF16 = mybir.dt.bfloat16
E4 = ml_dtypes.float8_e4m3
DR = mybir.MatmulPerfMode.DoubleRow
ACOPY = mybir.ActivationFunctionType.Copy


def _r(ap):
    return ap.bitcast(F32R)

NCORES = 8
B = 64
BLOC = B // NCORES  # 8
IN_DIM = 2
UNITS = 64
CHX = BLOC * UNITS  # 512
C = CHX + BLOC * IN_DIM  # 528
CIN = BLOC * IN_DIM  # 16
CH = C // 2  # 264 (psum free-dim split)

# fp8 scales (powers of 2; value ranges deterministic for this problem)
SSC = 2.0**17   # supports scale (max |2^17 S| ~ 71 < 240)
A0 = 2.0**5     # x0 state scale (max |x0| ~ 5.4 -> 173)
AH = 2.0**8     # hop state scale (max |y| ~ 0.14 -> 36)
# PSUM after hop1 holds SSC*A0*y; after hops 2-4 SSC*AH*y.
M1 = AH / (SSC * A0)   # psum -> state mult, hop 1
MH = AH / (SSC * AH)   # psum -> state mult, hops 2-4
# projection: all terms tau-scaled in PSUM, descaled by the activation
TAU = 2.0**12   # fp8 m>=1 weights hold What*TAU/AH (range ~ [2e-3, 2.4])


def _build_nc(N, phases=("d0", "p0", "d1", "p1")):
    """Build the per-core Bass program (SPMD; same NEFF on all 8 cores).
    phases: subset/ordering of diffusion/projection passes (bench use)."""
    NB = N // 128  # row blocks (32 at full size)
    NP = NB // 2   # k-block pairs per contraction
    PCH = min(2048, N)  # phase-P n-chunk held in SBUF
    NHALF = N // PCH
    NFC = PCH // 512  # 512-wide proj chunks per PCH

    nc = bacc.Bacc("TRN2", target_bir_lowering=False, debug=False)

    # ---- external I/O ----
    x0pm = nc.dram_tensor("x0pm", [128, NB * C], F8, kind="ExternalInput").ap()
    stb = nc.dram_tensor("stb", [2, NB, 128, NB * 128], F8, kind="ExternalInput").ap()
    xint = nc.dram_tensor("xint", [CIN, N], BF16, kind="ExternalInput").ap()
    hxt = nc.dram_tensor("hxt", [BLOC, UNITS, N], BF16, kind="ExternalInput").ap()
    wfn = nc.dram_tensor("wfn", [66, 128], BF16, kind="ExternalInput").ap()
    wg = nc.dram_tensor("wg", [66, 64], BF16, kind="ExternalInput").ap()
    wfn8 = nc.dram_tensor("wfn8", [66, 4 * 128], F8, kind="ExternalInput").ap()
    wg8 = nc.dram_tensor("wg8", [66, 4 * 64], F8, kind="ExternalInput").ap()
    bfn = nc.dram_tensor("bfn", [128, 1], F32, kind="ExternalInput").ap()
    bg = nc.dram_tensor("bg", [64, 1], F32, kind="ExternalInput").ap()
    outt = nc.dram_tensor("outt", [BLOC, UNITS, N], F32, kind="ExternalOutput").ap()

    with tile.TileContext(nc) as tc, ExitStack() as ctx:
        # ---- persistent pools ----
        const = ctx.enter_context(tc.tile_pool(name="const", bufs=1))
        dram = ctx.enter_context(tc.tile_pool(name="dram", bufs=1, space="DRAM"))

        id8 = const.tile([128, 128], F8, name="id8")
        make_identity(nc, id8)
        wfn_sb = const.tile([66, 128], BF16, name="wfn_sb")
        nc.sync.dma_start(wfn_sb, wfn)
        wg_sb = const.tile([66, 64], BF16, name="wg_sb")
        nc.sync.dma_start(wg_sb, wg)
        wfn8_sb = const.tile([66, 4 * 128], F8, name="wfn8_sb")
        nc.sync.dma_start(wfn8_sb, wfn8)
        wg8_sb = const.tile([66, 4 * 64], F8, name="wg8_sb")
        nc.sync.dma_start(wg8_sb, wg8)
        bfn_sb = const.tile([128, 1], F32, name="bfn_sb")
        nc.sync.dma_start(bfn_sb, bfn)
        bg_sb = const.tile([64, 1], F32, name="bg_sb")
        nc.sync.dma_start(bg_sb, bg)
        # DRAM scratch: transposed fp8 diffusion results per gconv/hop, u
        # gate, rebuilt fp8 x0 for gconv2.
        # 640 = 5*128 rows: rows 0:512 hx-part, 512:528 input-part, rest pad
        # (padding lets each block spill as ONE 5x128x128 DMA).
        ytd = [
            [
                dram.tile([640, N], F8, name=f"ytd_{g}_{m}", tag=f"ytd_{g}_{m}")
                for m in range(1, 5)
            ]
            for g in range(2)
        ]
        yt0p = dram.tile([CHX, N], BF16, name="yt0p", tag="yt0p")
        ytin = dram.tile([4 * CIN, N], F8, name="ytin", tag="ytin")
        u_d = dram.tile([BLOC, UNITS, N], BF16, name="u_d", tag="u_d")

        # diffusion state buffers shared by both gconvs (d0's contents are
        # dead once its spills land; p0 rebuilds gconv2's x0 into bufA
        # in-place, no DRAM round-trip)
        yb = ctx.enter_context(tc.tile_pool(name="ybuf", bufs=1))
        bufA = yb.tile([128, NB * C], F8, name="bufA", tag="bufA")
        bufB = yb.tile([128, NB * C], F8, name="bufB", tag="bufB")

        uid_ctr = [0]

        def diffusion(g):
            """4 hops; X0 from x0pm for g=0; p0 pre-populates bufA for g=1."""
            uid_ctr[0] += 1
            u = uid_ctr[0]
            with (
                tc.tile_pool(name=f"s0r{g}_{u}", bufs=1) as s0p,
                tc.tile_pool(name=f"st{g}_{u}", bufs=3) as stp,
                tc.tile_pool(name=f"dps{g}_{u}", bufs=(3 if g == 0 else 4), space="PSUM") as dps,
                tc.tile_pool(name=f"tps{g}_{u}", bufs=(2 if g == 0 else 4), space="PSUM") as tps,
                tc.tile_pool(name=f"yts{g}_{u}", bufs=4) as ytsp,
            ):
                # S0 resident: hop 1 streams it in per-block; hop 2 reuses
                # it straight from SBUF (no DMA at all)
                s0r = s0p.tile([128, NB * NB * 128], F8, name=f"s0r{g}", tag="s0r")
                if g == 0:
                    q4 = NB * C // 4
                    for q in range(4):
                        nc.sync.dma_start(
                            bufA[:, q * q4 : (q + 1) * q4],
                            x0pm[:, q * q4 : (q + 1) * q4],
                        )

                # gconv2 skips the 16 input columns entirely: their diffusion
                # is identical to gconv1's, so phase P reuses g1's spills.
                W = C if g == 0 else CHX
                HW_ = W // 2  # 264 (g1) / 256 (g2) psum free split
                NJ = 5 if g == 0 else 4  # spill row-chunks

                def hop(src, dst, s_idx, yt_dst, m_idx, mult, s_mode="stream", s_eng=None):
                    # 4-block staged spill: HBM runs of 512B (vs 128B per
                    # block), 4x fewer+larger descriptors
                    grp = {}

                    def compute_block(nb):
                        # slab DMAs issued 2 blocks at a time (half the
                        # per-DMA fixed cost on the issuing queue)
                        if s_mode == "fill":
                            # fills ride the Pool queue: no upstream data deps,
                            # so they prefetch during the previous phase
                            if nb % 2 == 0:
                                nc.gpsimd.dma_start(
                                    s0r[
                                        :, nb * NB * 128 : (nb + 2) * NB * 128
                                    ].rearrange("p (b f) -> p b f", b=2),
                                    stb[s_idx, nb : nb + 2].rearrange("b p f -> p b f"),
                                )
                            slab = s0r[:, nb * NB * 128 : (nb + 1) * NB * 128]
                        elif s_mode == "reuse":
                            slab = s0r[:, nb * NB * 128 : (nb + 1) * NB * 128]
                        else:
                            if nb % 2 == 0:
                                grp["slab2t"] = stp.tile(
                                    [128, 2 * NB * 128], F8, name=f"slab{g}", tag="slab"
                                )
                                eng = (
                                    (nc.sync if (nb // 2) % 2 == 0 else nc.scalar)
                                    if s_eng == "alt"
                                    else (s_eng or nc.sync)
                                )
                                eng.dma_start(
                                    grp["slab2t"].rearrange("p (b f) -> p b f", b=2),
                                    stb[s_idx, nb : nb + 2].rearrange("b p f -> p b f"),
                                )
                            slab = grp["slab2t"][
                                :, (nb % 2) * NB * 128 : (nb % 2 + 1) * NB * 128
                            ]
                        slab2 = slab.rearrange("p (kp two f) -> p kp two f", two=2, f=128)
                        src2 = src.rearrange("p (kp two c) -> p kp two c", two=2, c=C)
                        if g == 0:
                            # 528 cols: two 264-wide psum groups (>512 limit)
                            pa = dps.tile([128, HW_], F32, name=f"pa{g}", tag="pa")
                            pb = dps.tile([128, HW_], F32, name=f"pb{g}", tag="pb")
                            for kp in range(NP):
                                lh = slab2[:, kp]
                                nc.tensor.matmul(
                                    pa,
                                    lh,
                                    src2[:, kp, :, 0:HW_],
                                    start=(kp == 0),
                                    stop=(kp == NP - 1),
                                    perf_mode=DR,
                                )
                                nc.tensor.matmul(
                                    pb,
                                    lh,
                                    src2[:, kp, :, HW_:W],
                                    start=(kp == 0),
                                    stop=(kp == NP - 1),
                                    perf_mode=DR,
                                )
                            # state requant on ACT (keeps DVE for transposes)
                            nc.scalar.activation(
                                dst[:, nb * C : nb * C + HW_], pa, ACOPY, scale=mult
                            )
                            nc.vector.tensor_scalar_mul(
                                dst[:, nb * C + HW_ : nb * C + W], pb, mult
                            )
                        else:
                            # 512 cols fit one psum bank: single matmul per
                            # pair -> half the weight loads
                            pa = dps.tile([128, W], F32, name=f"pa{g}", tag="pa")
                            for kp in range(NP):
                                nc.tensor.matmul(
                                    pa,
                                    slab2[:, kp],
                                    src2[:, kp, :, 0:W],
                                    start=(kp == 0),
                                    stop=(kp == NP - 1),
                                    perf_mode=DR,
                                )
                            nc.scalar.activation(
                                dst[:, nb * C : nb * C + W], pa, ACOPY, scale=mult
                            )

                    def transpose_block(nb):
                        # transpose the block's columns into the group's
                        # staging tile; spill 4 blocks at once.  fp8 PE
                        # transpose must write PSUM with element step 2.
                        qb = nb % 4
                        if qb == 0:
                            grp["yts"] = ytsp.tile(
                                [128, NJ, 4, 128], F8, name=f"yts{g}", tag="yts"
                            )
                        yts = grp["yts"]
                        # pairs of transposes share one psum tile -> one DVE
                        # copy per pair (halves PSUM-read latency overhead)
                        for jp in range(2):
                            tpp = tps.tile(
                                [128, 2, 128, 2], F8, name=f"tpp{g}", tag="tpp"
                            )
                            for jj in range(2):
                                j = 2 * jp + jj
                                nc.tensor.transpose(
                                    tpp[:, jj, :, 0],
                                    dst[:, nb * C + j * 128 : nb * C + (j + 1) * 128],
                                    id8,
                                )
                            # one staging copy per pair split DVE/ACT so
                            # neither engine paces the spill path
                            if jp == 0:
                                nc.vector.tensor_copy(
                                    yts[:, 0:2, qb, :], tpp[:, :, :, 0]
                                )
                            else:
                                nc.scalar.activation(
                                    yts[:, 2:4, qb, :], tpp[:, :, :, 0], ACOPY
                                )
                        if g == 0:
                            tpi = tps.tile([128, 2, 128, 2], F8, name=f"tpi{g}", tag="tpp")
                            nc.tensor.transpose(
                                tpi[:CIN, 0, :, 0],
                                dst[:, nb * C + CHX : (nb + 1) * C],
                                id8,
                            )
                            nc.vector.tensor_copy(yts[:CIN, 4, qb, :], tpi[:CIN, 0, :, 0])
                        if qb == 3:
                            cols = slice((nb - 3) * 128, (nb + 1) * 128)
                            nc.gpsimd.dma_start(
                                yt_dst[:512, cols].rearrange("(j r) n -> r j n", r=128),
                                yts[:, 0:4].rearrange("p j q n -> p j (q n)"),
                            )
                            if g == 0:
                                nc.gpsimd.dma_start(
                                    ytin[m_idx * CIN : (m_idx + 1) * CIN, cols],
                                    yts[:CIN, 4].rearrange("p q n -> p (q n)"),
                                )

                    # transposes deferred by 2 blocks so PE never stalls on
                    # the DVE psum-copies feeding them
                    for nb in range(NB):
                        compute_block(nb)
                        if nb >= 2:
                            transpose_block(nb - 2)
                    transpose_block(NB - 2)
                    transpose_block(NB - 1)

                hop(bufA, bufB, 0, ytd[g][0], 0, M1, "fill")   # y1 = S0 @ y0
                hop(bufB, bufA, 0, ytd[g][1], 1, MH, "reuse")  # y2 = S0 @ y1
                hop(bufB, bufA, 1, ytd[g][2], 2, MH)  # y3 = S1 @ y1 (y2 spilled)
                # hop 4's slab chunks alternate sync/ACT: sync drains its
                # share early, so the next projection's loads (queued right
                # behind) prefetch while this hop is still computing
                hop(bufA, bufB, 1, ytd[g][3], 3, MH, s_eng="alt")  # y4 = S1 @ y3

        def projection(g):
            D = 128 if g == 0 else 64
            w_sb = wfn_sb if g == 0 else wg_sb
            w8_sb = wfn8_sb if g == 0 else wg8_sb
            uid_ctr[0] += 1
            u = uid_ctr[0]
            pctx = {}
            with (
                tc.tile_pool(name=f"ytp{g}_{u}", bufs=12) as ytp,
                tc.tile_pool(name=f"aux{g}_{u}", bufs=6) as aux,
                tc.tile_pool(
                    name=f"zps{g}_{u}", bufs=(4 if g == 0 else 6), space="PSUM"
                ) as zps,
                tc.tile_pool(name=f"tpq{g}_{u}", bufs=3, space="PSUM") as tpq,
            ):
                # half-outer: the first half's spill inputs are complete
                # mid-hop4, so its loads (already queued on the free sync
                # queue) stream in while hop 4 is still computing
                for half in range(NHALF):
                    for b in range(BLOC):
                        ns = half * PCH
                        if g == 1:
                            hx_t = aux.tile(
                                [UNITS, PCH], BF16, name=f"hx_t{g}", tag="hx_t", bufs=4
                            )
                            nc.scalar.dma_start(hx_t, hxt[b, :, ns : ns + PCH])
                            u_t = aux.tile([UNITS, PCH], BF16, name="u_t", tag="u_t", bufs=4)
                            nc.gpsimd.dma_start(u_t, u_d[b, :, ns : ns + PCH])
                        # m = 0 tile: bf16
                        yt0 = ytp.tile([66, PCH], BF16, name=f"yt0_{g}", tag="yt0", bufs=5)
                        hx_src = (
                            hxt[b, :, ns : ns + PCH]
                            if g == 0
                            else yt0p[b * UNITS : (b + 1) * UNITS, ns : ns + PCH]
                        )
                        nc.sync.dma_start(yt0[0:UNITS, :], hx_src)
                        nc.sync.dma_start(
                            yt0[UNITS:66, :], xint[b * 2 : b * 2 + 2, ns : ns + PCH]
                        )
                        if g == 0:
                            # group-batched outputs: u, r*hx (one DMA each)
                            u_acc = aux.tile(
                                [UNITS, PCH], BF16, name="u_acc", tag="u_acc", bufs=3
                            )
                            rh_acc = aux.tile(
                                [UNITS, PCH], BF16, name="rh_acc", tag="rh_acc", bufs=3
                            )
                        else:
                            ot_acc = aux.tile(
                                [UNITS, PCH], F32, name="ot_acc", tag="ot_acc", bufs=3
                            )
                        # m = 1..4 tiles: fp8 spills, pair-packed for
                        # DoubleRow projection matmuls
                        yts = []
                        for p in range(2):
                            yt_t = ytp.tile(
                                [66, 2, PCH], F8, name=f"yt{g}", tag="yt", bufs=10
                            )
                            for half2 in range(2):
                                m = 2 * p + half2 + 1
                                ytm = ytd[g][m - 1]
                                hx_src8 = ytm[b * UNITS : (b + 1) * UNITS, ns : ns + PCH]
                                eng = nc.sync if m % 2 == 0 else nc.scalar
                                eng.dma_start(yt_t[0:UNITS, half2, :], hx_src8)
                            # both members' input rows in one strided DMA
                            (nc.sync if p == 0 else nc.scalar).dma_start(
                                yt_t[UNITS:66, :, :],
                                ytin.rearrange("(m r) n -> m r n", r=CIN)[
                                    2 * p : 2 * p + 2,
                                    b * 2 : b * 2 + 2,
                                    ns : ns + PCH,
                                ].rearrange("m r n -> r m n"),
                            )
                            yts.append(yt_t)
                        w8p = w8_sb.rearrange("f (p two d) -> f p two d", p=2, two=2)
                        for nfc in range(NFC):
                            zp = zps.tile([D, 512], F32, name=f"zp{g}", tag="zp")
                            nc.tensor.matmul(
                                zp,
                                w_sb,
                                yt0[:, nfc * 512 : (nfc + 1) * 512],
                                start=True,
                                stop=False,
                            )
                            for p in range(2):
                                nc.tensor.matmul(
                                    zp,
                                    w8p[:, p],
                                    yts[p][:, :, nfc * 512 : (nfc + 1) * 512],
                                    start=False,
                                    stop=(p == 1),
                                    perf_mode=DR,
                                )
                            nf0 = ns + nfc * 512
                            if g == 0:
                                val = aux.tile([128, 512], BF16, name="val", tag="val")
                                nc.scalar.activation(
                                    val,
                                    zp,
                                    mybir.ActivationFunctionType.Sigmoid,
                                    bias=bfn_sb,
                                    scale=1.0 / TAU,
                                )
                                rh = rh_acc[:, nfc * 512 : (nfc + 1) * 512]
                                nc.vector.tensor_mul(
                                    rh,
                                    val[0:64, :],
                                    yt0[0:UNITS, nfc * 512 : (nfc + 1) * 512],
                                )
                                nc.vector.tensor_copy(
                                    u_acc[:, nfc * 512 : (nfc + 1) * 512], val[64:128, :]
                                )
                                # un-transpose r*hx (fp8, x2^5) into gconv2's
                                # diffusion layout; spill chunk-pairs (512B
                                # HBM runs)
                                rh8 = aux.tile([64, 512], F8, name="rh8", tag="rh8")
                                nc.vector.tensor_scalar_mul(rh8, rh, A0)
                                if nfc % 2 == 0:
                                    pctx["xs4"] = aux.tile(
                                        [128, 8, 64], F8, name="xs4", tag="xs4"
                                    )
                                xs4 = pctx["xs4"]
                                # transpose pairs share one psum tile -> one
                                # DVE copy per pair
                                for sp in range(2):
                                    tpp = tpq.tile(
                                        [128, 2, 64, 2], F8, name="tpq_t", tag="tpq"
                                    )
                                    for ss in range(2):
                                        sub = 2 * sp + ss
                                        nc.tensor.transpose(
                                            tpp[:, ss, :, 0],
                                            rh8[:, sub * 128 : (sub + 1) * 128],
                                            id8[0:64, 0:64],
                                        )
                                    nc.vector.tensor_copy(
                                        xs4[:, (nfc % 2) * 4 + 2 * sp : (nfc % 2) * 4 + 2 * sp + 2, :],
                                        tpp[:, :, :, 0],
                                    )
                                if nfc % 2 == 1:
                                    kb0 = (nf0 - 512) // 128
                                    nc.scalar.dma_start(
                                        bufA.rearrange("p (k c) -> p k c", c=C)[
                                            :, kb0 : kb0 + 8,
                                            b * UNITS : (b + 1) * UNITS,
                                        ],
                                        xs4,
                                    )
                            else:
                                ct = aux.tile([64, 512], BF16, name="ct", tag="ct")
                                nc.scalar.activation(
                                    ct,
                                    zp,
                                    mybir.ActivationFunctionType.Tanh,
                                    bias=bg_sb,
                                    scale=1.0 / TAU,
                                )
                                tmp = aux.tile([64, 512], BF16, name="tmp", tag="tmp")
                                nc.vector.tensor_sub(
                                    tmp, hx_t[:, nfc * 512 : (nfc + 1) * 512], ct
                                )
                                nc.vector.tensor_mul(
                                    tmp, tmp, u_t[:, nfc * 512 : (nfc + 1) * 512]
                                )
                                nc.vector.tensor_add(
                                    ot_acc[:, nfc * 512 : (nfc + 1) * 512], tmp, ct
                                )
                        if g == 0:
                            nc.sync.dma_start(u_d[b, :, ns : ns + PCH], u_acc)
                            nc.gpsimd.dma_start(
                                yt0p[b * UNITS : (b + 1) * UNITS, ns : ns + PCH], rh_acc
                            )
                        else:
                            # alternate queues so stores don't head-block the
                            # next group's loads (FIFO per queue)
                            eng_o = nc.sync if b % 2 == 0 else nc.gpsimd
                            eng_o.dma_start(outt[b, :, ns : ns + PCH], ot_acc)

        for ph in phases:
            {"d0": lambda: diffusion(0), "p0": lambda: projection(0),
             "d1": lambda: diffusion(1), "p1": lambda: projection(1)}[ph]()

    nc.compile()
    return nc


def _fold_weights(w, out_dim):
    """w: (330, out). Returns [5, 66, out] with the reference's x0c-mutation
    linear combinations folded in and rows reordered hx-first."""
    Wm = w.reshape(66, 5, out_dim)
    What = np.stack(
        [
            Wm[:, 0] - Wm[:, 2],
            Wm[:, 1] - Wm[:, 4],
            2.0 * Wm[:, 2],
            Wm[:, 3],
            2.0 * Wm[:, 4],
        ]
    )  # [5, 66, out]
    What = np.concatenate([What[:, 2:, :], What[:, :2, :]], axis=1)  # hx rows first
    return What.astype(np.float32)


def _q8(x, scale):
    return np.clip(x.astype(np.float32) * scale, -240.0, 240.0).astype(E4)


_NC_CACHE = {}


def _get_nc(N):
    if N not in _NC_CACHE:
        _NC_CACHE[N] = _build_nc(N)
    return _NC_CACHE[N]


def _prepare_in_maps(inputs, hx, supports, w_fn, b_fn, w_g, b_g):
    inputs = np.ascontiguousarray(np.asarray(inputs), dtype=np.float32)
    hx = np.ascontiguousarray(np.asarray(hx), dtype=np.float32)
    supports = np.ascontiguousarray(np.asarray(supports), dtype=np.float32)
    w_fn = np.asarray(w_fn, dtype=np.float32)
    b_fn = np.asarray(b_fn, dtype=np.float32)
    w_g = np.asarray(w_g, dtype=np.float32)
    b_g = np.asarray(b_g, dtype=np.float32)

    N = supports.shape[1]
    NB = N // 128

    # ---- replicated tensors ----
    # stb[s, nb, kp, kb*128+m] = supports[s][nb*128+m, kb*128+kp], fp8 x 2^17
    stb = _q8(
        np.ascontiguousarray(
            supports.reshape(2, NB, 128, NB, 128).transpose(0, 1, 4, 3, 2)
        ).reshape(2, NB, 128, NB * 128),
        SSC,
    )
    WhatF = _fold_weights(w_fn, 128)  # [5, 66, 128]
    WhatG = _fold_weights(w_g, 64)
    # all projection terms tau-scaled in PSUM (activation descales)
    wfn_h = np.ascontiguousarray(WhatF[0] * TAU).astype(ml_dtypes.bfloat16)
    wg_h = np.ascontiguousarray(WhatG[0] * TAU).astype(ml_dtypes.bfloat16)
    # m>=1 weights: fp8, tau-scaled and descaled by the hop state scale
    wfn8_h = _q8(
        np.ascontiguousarray((WhatF[1:] / AH).transpose(1, 0, 2).reshape(66, 4 * 128)),
        TAU,
    )
    wg8_h = _q8(
        np.ascontiguousarray((WhatG[1:] / AH).transpose(1, 0, 2).reshape(66, 4 * 64)),
        TAU,
    )
    bfn_h = b_fn.reshape(128, 1).copy()
    bg_h = b_g.reshape(64, 1).copy()

    in_maps = []
    for c in range(NCORES):
        sl = slice(c * BLOC, (c + 1) * BLOC)
        inp_c = inputs[sl].reshape(BLOC, N, IN_DIM)
        hx_c = hx[sl].reshape(BLOC, N, UNITS)
        # X0 [N, 528]: hx cols b*64+u, input cols 512 + b*2 + j
        x0 = np.concatenate(
            [
                hx_c.transpose(1, 0, 2).reshape(N, CHX),
                inp_c.transpose(1, 0, 2).reshape(N, CIN),
            ],
            axis=1,
        )
        x0pm = _q8(
            np.ascontiguousarray(x0.reshape(NB, 128, C).transpose(1, 0, 2)).reshape(
                128, NB * C
            ),
            A0,
        )
        xin = x0[:, CHX:]
        xint = np.ascontiguousarray(xin.T).astype(ml_dtypes.bfloat16)
        hxt = np.ascontiguousarray(hx_c.transpose(0, 2, 1)).astype(ml_dtypes.bfloat16)
        in_maps.append(
            {
                "x0pm": x0pm,
                "stb": stb,
                "xint": xint,
                "hxt": hxt,
                "wfn": wfn_h,
                "wg": wg_h,
                "wfn8": wfn8_h,
                "wg8": wg8_h,
                "bfn": bfn_h,
                "bg": bg_h,
            }
        )

    return in_maps


def kernel(inputs, hx, supports, w_fn, b_fn, w_g, b_g):
    N = np.asarray(supports).shape[1]
    nc = _get_nc(N)
    in_maps = _prepare_in_maps(inputs, hx, supports, w_fn, b_fn, w_g, b_g)
    kernel.last_in_maps = in_maps
    res = run_bass_kernel_spmd(
        nc,
        in_maps,
        core_ids=list(range(NCORES)),
        trace=bool(int(os.environ.get("DCGRU_TRACE", "0"))),
    )

    out = np.empty((B, N * UNITS), np.float32)
    for c in range(NCORES):
        outt = res.results[c]["outt"]  # [BLOC, UNITS, N]
        out[c * BLOC : (c + 1) * BLOC] = outt.transpose(0, 2, 1).reshape(BLOC, -1)
    kernel.last_results = res
    return out


# revision 91
# speedup vs baseline: 1.0163x; 1.0163x over previous
"""DCGRU cell (DCRNN) Trainium2 Bass kernel — fp8 diffusion version.

Strategy: data-parallel over batch B=64 across 8 NeuronCores (8 batches per
core); supports + gconv weights replicated.

Math restructuring (validated in numpy against the jax reference):
  reference diffusion xs = [x0, S0@x0, 2*S0^2@x0 - x0, S1@S0@x0, 2*S1^2@S0@x0 - S0@x0]
  -> raw chain     ys = [y0, y1=S0@y0, y2=S0@y1, y3=S1@y1, y4=S1@y3]
  with the 2a-b combinations folded into the projection weights on the host:
  What = [W0-W2, W1-W4, 2*W2, W3, 2*W4] (Wm = rows insz*5+m of the gconv W).

fp8 acceleration: the diffusion hops run as float8e4 DoubleRow matmuls
(double-pumped PE: 2 fp8 rows/cycle, 256-deep contraction per instruction;
measured ~4x fp32r throughput on HW) with 4x less support HBM traffic.
Supports are host-quantized to e4m3 scaled by 2^17 (entries in [0.06, 71] —
all normal range); X-states are quantized on-device with per-hop power-of-2
scales (x0: 2^5, hops: 2^8) applied during the PSUM->SBUF requant copy (ACT).
Hop-term quantization is benign because the projection is dominated by its
m=0 term (hop outputs are ~50x smaller).  Measured end-to-end rel err 3.3e-3
vs the fp32 reference (harness gate 2e-2).

Per-core device layout:
  Diffusion state X [N, C=528] fp8 in SBUF, columns c = b*64+u (hx part) then
  512 + b*2 + j (input part); bufA/bufB are shared by both gconvs (p0
  rebuilds gconv2's x0 into bufA in place via the xs4 un-transpose, no DRAM
  round trip).  S0 is streamed once per gconv into a resident SBUF tile
  (hop 2 reuses it with no DMA); S1 is streamed per hop in 2-block chunks.
  After each hop the fp8 state is PE-transposed (step-2 PSUM out, paired
  tiles) and spilled to DRAM as YT [640, N] fp8 in 4-block groups (512B HBM
  runs).  Projection: ZT_b[out,n] = sum_m What_m.T @ YT_m[b-rows, n]
  accumulated tau-scaled (2^12) in one PSUM group: m=0 bf16 x bf16, m=1..4
  as two DoubleRow fp8 pairs (weights hold What*TAU/AH); the activation
  (sigmoid/tanh) descales via its scale operand and adds the bias.  Gate
  tensors (hx, u, r*hx) travel as bf16; gate arithmetic is split across
  DVE/Pool; group outputs are batched into single DMAs.  Host un-transposes
  the final output during unsharding.
"""

import os
from contextlib import ExitStack

import numpy as np
import ml_dtypes

import concourse.bacc as bacc
import concourse.mybir as mybir
import concourse.tile as tile
from concourse.bass_utils import run_bass_kernel_spmd
from concourse.masks import make_identity

F32 = mybir.dt.float32
F32R = mybir.dt.float32r
F8 = mybir.dt.float8e4
BF16 = mybir.dt.bfloat16
E4 = ml_dtypes.float8_e4m3
DR = mybir.MatmulPerfMode.DoubleRow
ACOPY = mybir.ActivationFunctionType.Copy


def _r(ap):
    return ap.bitcast(F32R)

NCORES = 8
B = 64
BLOC = B // NCORES  # 8
IN_DIM = 2
UNITS = 64
CHX = BLOC * UNITS  # 512
C = CHX + BLOC * IN_DIM  # 528
CIN = BLOC * IN_DIM  # 16
CH = C // 2  # 264 (psum free-dim split)

# fp8 scales (powers of 2; value ranges deterministic for this problem)
SSC = 2.0**17   # supports scale (max |2^17 S| ~ 71 < 240)
A0 = 2.0**5     # x0 state scale (max |x0| ~ 5.4 -> 173)
AH = 2.0**8     # hop state scale (max |y| ~ 0.14 -> 36)
# PSUM after hop1 holds SSC*A0*y; after hops 2-4 SSC*AH*y.
M1 = AH / (SSC * A0)   # psum -> state mult, hop 1
MH = AH / (SSC * AH)   # psum -> state mult, hops 2-4
# projection: all terms tau-scaled in PSUM, descaled by the activation
TAU = 2.0**12   # fp8 m>=1 weights hold What*TAU/AH (range ~ [2e-3, 2.4])


def _build_nc(N, phases=("d0", "p0", "d1", "p1")):
    """Build the per-core Bass program (SPMD; same NEFF on all 8 cores).
    phases: subset/ordering of diffusion/projection passes (bench use)."""
    NB = N // 128  # row blocks (32 at full size)
    NP = NB // 2   # k-block pairs per contraction
    PCH = min(2048, N)  # phase-P n-chunk held in SBUF
    NHALF = N // PCH
    NFC = PCH // 512  # 512-wide proj chunks per PCH

    nc = bacc.Bacc("TRN2", target_bir_lowering=False, debug=False)

    # ---- external I/O ----
    x0pm = nc.dram_tensor("x0pm", [128, NB * C], F8, kind="ExternalInput").ap()
    stb = nc.dram_tensor("stb", [2, NB, 128, NB * 128], F8, kind="ExternalInput").ap()
    xint = nc.dram_tensor("xint", [CIN, N], BF16, kind="ExternalInput").ap()
    hxt = nc.dram_tensor("hxt", [BLOC, UNITS, N], BF16, kind="ExternalInput").ap()
    wfn = nc.dram_tensor("wfn", [66, 128], BF16, kind="ExternalInput").ap()
    wg = nc.dram_tensor("wg", [66, 64], BF16, kind="ExternalInput").ap()
    wfn8 = nc.dram_tensor("wfn8", [66, 4 * 128], F8, kind="ExternalInput").ap()
    wg8 = nc.dram_tensor("wg8", [66, 4 * 64], F8, kind="ExternalInput").ap()
    bfn = nc.dram_tensor("bfn", [128, 1], F32, kind="ExternalInput").ap()
    bg = nc.dram_tensor("bg", [64, 1], F32, kind="ExternalInput").ap()
    outt = nc.dram_tensor("outt", [BLOC, UNITS, N], F32, kind="ExternalOutput").ap()

    with tile.TileContext(nc) as tc, ExitStack() as ctx:
        # ---- persistent pools ----
        const = ctx.enter_context(tc.tile_pool(name="const", bufs=1))
        dram = ctx.enter_context(tc.tile_pool(name="dram", bufs=1, space="DRAM"))

        id8 = const.tile([128, 128], F8, name="id8")
        make_identity(nc, id8)
        wfn_sb = const.tile([66, 128], BF16, name="wfn_sb")
        nc.sync.dma_start(wfn_sb, wfn)
        wg_sb = const.tile([66, 64], BF16, name="wg_sb")
        nc.sync.dma_start(wg_sb, wg)
        wfn8_sb = const.tile([66, 4 * 128], F8, name="wfn8_sb")
        nc.sync.dma_start(wfn8_sb, wfn8)
        wg8_sb = const.tile([66, 4 * 64], F8, name="wg8_sb")
        nc.sync.dma_start(wg8_sb, wg8)
        bfn_sb = const.tile([128, 1], F32, name="bfn_sb")
        nc.sync.dma_start(bfn_sb, bfn)
        bg_sb = const.tile([64, 1], F32, name="bg_sb")
        nc.sync.dma_start(bg_sb, bg)
        # DRAM scratch: transposed fp8 diffusion results per gconv/hop, u
        # gate, rebuilt fp8 x0 for gconv2.
        # 640 = 5*128 rows: rows 0:512 hx-part, 512:528 input-part, rest pad
        # (padding lets each block spill as ONE 5x128x128 DMA).
        ytd = [
            [
                dram.tile([640, N], F8, name=f"ytd_{g}_{m}", tag=f"ytd_{g}_{m}")
                for m in range(1, 5)
            ]
            for g in range(2)
        ]
        yt0p = dram.tile([CHX, N], BF16, name="yt0p", tag="yt0p")
        ytin = dram.tile([4 * CIN, N], F8, name="ytin", tag="ytin")
        u_d = dram.tile([BLOC, UNITS, N], BF16, name="u_d", tag="u_d")

        # diffusion state buffers shared by both gconvs (d0's contents are
        # dead once its spills land; p0 rebuilds gconv2's x0 into bufA
        # in-place, no DRAM round-trip)
        yb = ctx.enter_context(tc.tile_pool(name="ybuf", bufs=1))
        bufA = yb.tile([128, NB * C], F8, name="bufA", tag="bufA")
        bufB = yb.tile([128, NB * C], F8, name="bufB", tag="bufB")

        uid_ctr = [0]

        def diffusion(g):
            """4 hops; X0 from x0pm for g=0; p0 pre-populates bufA for g=1."""
            uid_ctr[0] += 1
            u = uid_ctr[0]
            with (
                tc.tile_pool(name=f"s0r{g}_{u}", bufs=1) as s0p,
                tc.tile_pool(name=f"st{g}_{u}", bufs=3) as stp,
                tc.tile_pool(name=f"dps{g}_{u}", bufs=(3 if g == 0 else 4), space="PSUM") as dps,
                tc.tile_pool(name=f"tps{g}_{u}", bufs=(2 if g == 0 else 4), space="PSUM") as tps,
                tc.tile_pool(name=f"yts{g}_{u}", bufs=4) as ytsp,
            ):
                # S0 resident: hop 1 streams it in per-block; hop 2 reuses
                # it straight from SBUF (no DMA at all)
                s0r = s0p.tile([128, NB * NB * 128], F8, name=f"s0r{g}", tag="s0r")
                if g == 0:
                    q4 = NB * C // 4
                    for q in range(4):
                        nc.sync.dma_start(
                            bufA[:, q * q4 : (q + 1) * q4],
                            x0pm[:, q * q4 : (q + 1) * q4],
                        )

                # gconv2 skips the 16 input columns entirely: their diffusion
                # is identical to gconv1's, so phase P reuses g1's spills.
                W = C if g == 0 else CHX
                HW_ = W // 2  # 264 (g1) / 256 (g2) psum free split
                NJ = 5 if g == 0 else 4  # spill row-chunks

                def hop(src, dst, s_idx, yt_dst, m_idx, mult, s_mode="stream", s_eng=None):
                    # 4-block staged spill: HBM runs of 512B (vs 128B per
                    # block), 4x fewer+larger descriptors
                    grp = {}

                    def compute_block(nb):
                        # slab DMAs issued 2 blocks at a time (half the
                        # per-DMA fixed cost on the issuing queue)
                        if s_mode == "fill":
                            # fills ride the Pool queue: no upstream data deps,
                            # so they prefetch during the previous phase
                            if nb % 2 == 0:
                                nc.gpsimd.dma_start(
                                    s0r[
                                        :, nb * NB * 128 : (nb + 2) * NB * 128
                                    ].rearrange("p (b f) -> p b f", b=2),
                                    stb[s_idx, nb : nb + 2].rearrange("b p f -> p b f"),
                                )
                            slab = s0r[:, nb * NB * 128 : (nb + 1) * NB * 128]
                        elif s_mode == "reuse":
                            slab = s0r[:, nb * NB * 128 : (nb + 1) * NB * 128]
                        else:
                            if nb % 2 == 0:
                                grp["slab2t"] = stp.tile(
                                    [128, 2 * NB * 128], F8, name=f"slab{g}", tag="slab"
                                )
                                eng = (
                                    (nc.sync if (nb // 2) % 2 == 0 else nc.scalar)
                                    if s_eng == "alt"
                                    else (s_eng or nc.sync)
                                )
                                eng.dma_start(
                                    grp["slab2t"].rearrange("p (b f) -> p b f", b=2),
                                    stb[s_idx, nb : nb + 2].rearrange("b p f -> p b f"),
                                )
                            slab = grp["slab2t"][
                                :, (nb % 2) * NB * 128 : (nb % 2 + 1) * NB * 128
                            ]
                        slab2 = slab.rearrange("p (kp two f) -> p kp two f", two=2, f=128)
                        src2 = src.rearrange("p (kp two c) -> p kp two c", two=2, c=C)
                        if g == 0:
                            # 528 cols: two 264-wide psum groups (>512 limit)
                            pa = dps.tile([128, HW_], F32, name=f"pa{g}", tag="pa")
                            pb = dps.tile([128, HW_], F32, name=f"pb{g}", tag="pb")
                            for kp in range(NP):
                                lh = slab2[:, kp]
                                nc.tensor.matmul(
                                    pa,
                                    lh,
                                    src2[:, kp, :, 0:HW_],
                                    start=(kp == 0),
                                    stop=(kp == NP - 1),
                                    perf_mode=DR,
                                )
                                nc.tensor.matmul(
                                    pb,
                                    lh,
                                    src2[:, kp, :, HW_:W],
                                    start=(kp == 0),
                                    stop=(kp == NP - 1),
                                    perf_mode=DR,
                                )
                            # state requant on ACT (keeps DVE for transposes)
                            nc.scalar.activation(
                                dst[:, nb * C : nb * C + HW_], pa, ACOPY, scale=mult
                            )
                            nc.vector.tensor_scalar_mul(
                                dst[:, nb * C + HW_ : nb * C + W], pb, mult
                            )
                        else:
                            # 512 cols fit one psum bank: single matmul per
                            # pair -> half the weight loads
                            pa = dps.tile([128, W], F32, name=f"pa{g}", tag="pa")
                            for kp in range(NP):
                                nc.tensor.matmul(
                                    pa,
                                    slab2[:, kp],
                                    src2[:, kp, :, 0:W],
                                    start=(kp == 0),
                                    stop=(kp == NP - 1),
                                    perf_mode=DR,
                                )
                            nc.scalar.activation(
                                dst[:, nb * C : nb * C + W], pa, ACOPY, scale=mult
                            )

                    def transpose_block(nb):
                        # transpose the block's columns into the group's
                        # staging tile; spill 4 blocks at once.  fp8 PE
                        # transpose must write PSUM with element step 2.
                        qb = nb % 4
                        if qb == 0:
                            grp["yts"] = ytsp.tile(
                                [128, NJ, 4, 128], F8, name=f"yts{g}", tag="yts"
                            )
                        yts = grp["yts"]
                        # pairs of transposes share one psum tile -> one DVE
                        # copy per pair (halves PSUM-read latency overhead)
                        for jp in range(2):
                            tpp = tps.tile(
                                [128, 2, 128, 2], F8, name=f"tpp{g}", tag="tpp"
                            )
                            for jj in range(2):
                                j = 2 * jp + jj
                                nc.tensor.transpose(
                                    tpp[:, jj, :, 0],
                                    dst[:, nb * C + j * 128 : nb * C + (j + 1) * 128],
                                    id8,
                                )
                            # one staging copy per pair split DVE/ACT so
                            # neither engine paces the spill path
                            if jp == 0:
                                nc.vector.tensor_copy(
                                    yts[:, 0:2, qb, :], tpp[:, :, :, 0]
                                )
                            else:
                                nc.scalar.activation(
                                    yts[:, 2:4, qb, :], tpp[:, :, :, 0], ACOPY
                                )
                        if g == 0:
                            tpi = tps.tile([128, 2, 128, 2], F8, name=f"tpi{g}", tag="tpp")
                            nc.tensor.transpose(
                                tpi[:CIN, 0, :, 0],
                                dst[:, nb * C + CHX : (nb + 1) * C],
                                id8,
                            )
                            nc.vector.tensor_copy(yts[:CIN, 4, qb, :], tpi[:CIN, 0, :, 0])
                        if qb == 3:
                            cols = slice((nb - 3) * 128, (nb + 1) * 128)
                            nc.gpsimd.dma_start(
                                yt_dst[:512, cols].rearrange("(j r) n -> r j n", r=128),
                                yts[:, 0:4].rearrange("p j q n -> p j (q n)"),
                            )
                            if g == 0:
                                nc.gpsimd.dma_start(
                                    ytin[m_idx * CIN : (m_idx + 1) * CIN, cols],
                                    yts[:CIN, 4].rearrange("p q n -> p (q n)"),
                                )

                    # transposes deferred by 2 blocks so PE never stalls on
                    # the DVE psum-copies feeding them
                    for nb in range(NB):
                        compute_block(nb)
                        if nb >= 2:
                            transpose_block(nb - 2)
                    transpose_block(NB - 2)
                    transpose_block(NB - 1)

                hop(bufA, bufB, 0, ytd[g][0], 0, M1, "fill")   # y1 = S0 @ y0
                hop(bufB, bufA, 0, ytd[g][1], 1, MH, "reuse")  # y2 = S0 @ y1
                hop(bufB, bufA, 1, ytd[g][2], 2, MH)  # y3 = S1 @ y1 (y2 spilled)
                # hop 4's slab chunks alternate sync/ACT: sync drains its
                # share early, so the next projection's loads (queued right
                # behind) prefetch while this hop is still computing
                hop(bufA, bufB, 1, ytd[g][3], 3, MH, s_eng="alt")  # y4 = S1 @ y3

        def projection(g):
            D = 128 if g == 0 else 64
            w_sb = wfn_sb if g == 0 else wg_sb
            w8_sb = wfn8_sb if g == 0 else wg8_sb
            uid_ctr[0] += 1
            u = uid_ctr[0]
            pctx = {}
            with (
                tc.tile_pool(name=f"ytp{g}_{u}", bufs=12) as ytp,
                tc.tile_pool(name=f"aux{g}_{u}", bufs=6) as aux,
                tc.tile_pool(
                    name=f"zps{g}_{u}", bufs=(4 if g == 0 else 6), space="PSUM"
                ) as zps,
                tc.tile_pool(name=f"tpq{g}_{u}", bufs=3, space="PSUM") as tpq,
            ):
                # half-outer: the first half's spill inputs are complete
                # mid-hop4, so its loads (already queued on the free sync
                # queue) stream in while hop 4 is still computing
                for half in range(NHALF):
                    for b in range(BLOC):
                        ns = half * PCH
                        if g == 1:
                            hx_t = aux.tile(
                                [UNITS, PCH], BF16, name=f"hx_t{g}", tag="hx_t", bufs=4
                            )
                            nc.scalar.dma_start(hx_t, hxt[b, :, ns : ns + PCH])
                            u_t = aux.tile([UNITS, PCH], BF16, name="u_t", tag="u_t", bufs=4)
                            nc.gpsimd.dma_start(u_t, u_d[b, :, ns : ns + PCH])
                        # m = 0 tile: bf16
                        yt0 = ytp.tile([66, PCH], BF16, name=f"yt0_{g}", tag="yt0", bufs=5)
                        hx_src = (
                            hxt[b, :, ns : ns + PCH]
                            if g == 0
                            else yt0p[b * UNITS : (b + 1) * UNITS, ns : ns + PCH]
                        )
                        nc.sync.dma_start(yt0[0:UNITS, :], hx_src)
                        nc.sync.dma_start(
                            yt0[UNITS:66, :], xint[b * 2 : b * 2 + 2, ns : ns + PCH]
                        )
                        if g == 0:
                            # group-batched outputs: u, r*hx (one DMA each)
                            u_acc = aux.tile(
                                [UNITS, PCH], BF16, name="u_acc", tag="u_acc", bufs=3
                            )
                            rh_acc = aux.tile(
                                [UNITS, PCH], BF16, name="rh_acc", tag="rh_acc", bufs=3
                            )
                        else:
                            ot_acc = aux.tile(
                                [UNITS, PCH], F32, name="ot_acc", tag="ot_acc", bufs=3
                            )
                        # m = 1..4 tiles: fp8 spills, pair-packed for
                        # DoubleRow projection matmuls
                        yts = []
                        for p in range(2):
                            yt_t = ytp.tile(
                                [66, 2, PCH], F8, name=f"yt{g}", tag="yt", bufs=10
                            )
                            for half2 in range(2):
                                m = 2 * p + half2 + 1
                                ytm = ytd[g][m - 1]
                                hx_src8 = ytm[b * UNITS : (b + 1) * UNITS, ns : ns + PCH]
                                eng = nc.sync if m % 2 == 0 else nc.scalar
                                eng.dma_start(yt_t[0:UNITS, half2, :], hx_src8)
                            # both members' input rows in one strided DMA
                            (nc.sync if p == 0 else nc.scalar).dma_start(
                                yt_t[UNITS:66, :, :],
                                ytin.rearrange("(m r) n -> m r n", r=CIN)[
                                    2 * p : 2 * p + 2,
                                    b * 2 : b * 2 + 2,
                                    ns : ns + PCH,
                                ].rearrange("m r n -> r m n"),
                            )
                            yts.append(yt_t)
                        w8p = w8_sb.rearrange("f (p two d) -> f p two d", p=2, two=2)
                        for nfc in range(NFC):
                            zp = zps.tile([D, 512], F32, name=f"zp{g}", tag="zp")
                            nc.tensor.matmul(
                                zp,
                                w_sb,
                                yt0[:, nfc * 512 : (nfc + 1) * 512],
                                start=True,
                                stop=False,
                            )
                            for p in range(2):
                                nc.tensor.matmul(
                                    zp,
                                    w8p[:, p],
                                    yts[p][:, :, nfc * 512 : (nfc + 1) * 512],
                                    start=False,
                                    stop=(p == 1),
                                    perf_mode=DR,
                                )
                            nf0 = ns + nfc * 512
                            if g == 0:
                                val = aux.tile([128, 512], BF16, name="val", tag="val")
                                nc.scalar.activation(
                                    val,
                                    zp,
                                    mybir.ActivationFunctionType.Sigmoid,
                                    bias=bfn_sb,
                                    scale=1.0 / TAU,
                                )
                                rh = rh_acc[:, nfc * 512 : (nfc + 1) * 512]
                                nc.vector.tensor_mul(
                                    rh,
                                    val[0:64, :],
                                    yt0[0:UNITS, nfc * 512 : (nfc + 1) * 512],
                                )
                                nc.vector.tensor_copy(
                                    u_acc[:, nfc * 512 : (nfc + 1) * 512], val[64:128, :]
                                )
                                # un-transpose r*hx (fp8, x2^5) into gconv2's
                                # diffusion layout; spill chunk-pairs (512B
                                # HBM runs)
                                rh8 = aux.tile([64, 512], F8, name="rh8", tag="rh8")
                                nc.vector.tensor_scalar_mul(rh8, rh, A0)
                                if nfc % 2 == 0:
                                    pctx["xs4"] = aux.tile(
                                        [128, 8, 64], F8, name="xs4", tag="xs4"
                                    )
                                xs4 = pctx["xs4"]
                                # transpose pairs share one psum tile -> one
                                # DVE copy per pair
                                for sp in range(2):
                                    tpp = tpq.tile(
                                        [128, 2, 64, 2], F8, name="tpq_t", tag="tpq"
                                    )
                                    for ss in range(2):
                                        sub = 2 * sp + ss
                                        nc.tensor.transpose(
                                            tpp[:, ss, :, 0],
                                            rh8[:, sub * 128 : (sub + 1) * 128],
                                            id8[0:64, 0:64],
                                        )
                                    nc.vector.tensor_copy(
                                        xs4[:, (nfc % 2) * 4 + 2 * sp : (nfc % 2) * 4 + 2 * sp + 2, :],
                                        tpp[:, :, :, 0],
                                    )
                                if nfc % 2 == 1:
                                    kb0 = (nf0 - 512) // 128
                                    nc.scalar.dma_start(
                                        bufA.rearrange("p (k c) -> p k c", c=C)[
                                            :, kb0 : kb0 + 8,
                                            b * UNITS : (b + 1) * UNITS,
                                        ],
                                        xs4,
                                    )
                            else:
                                ct = aux.tile([64, 512], BF16, name="ct", tag="ct")
                                nc.scalar.activation(
                                    ct,
                                    zp,
                                    mybir.ActivationFunctionType.Tanh,
                                    bias=bg_sb,
                                    scale=1.0 / TAU,
                                )
                                tmp = aux.tile([64, 512], BF16, name="tmp", tag="tmp")
                                nc.vector.tensor_sub(
                                    tmp, hx_t[:, nfc * 512 : (nfc + 1) * 512], ct
                                )
                                nc.vector.tensor_mul(
                                    tmp, tmp, u_t[:, nfc * 512 : (nfc + 1) * 512]
                                )
                                nc.vector.tensor_add(
                                    ot_acc[:, nfc * 512 : (nfc + 1) * 512], tmp, ct
                                )
                        if g == 0:
                            nc.sync.dma_start(u_d[b, :, ns : ns + PCH], u_acc)
                            nc.gpsimd.dma_start(
                                yt0p[b * UNITS : (b + 1) * UNITS, ns : ns + PCH], rh_acc
                            )
                        else:
                            # alternate queues so stores don't head-block the
                            # next group's loads (FIFO per queue)
                            eng_o = nc.sync if b % 2 == 0 else nc.gpsimd
                            eng_o.dma_start(outt[b, :, ns : ns + PCH], ot_acc)

        for ph in phases:
            {"d0": lambda: diffusion(0), "p0": lambda: projection(0),
             "d1": lambda: diffusion(1), "p1": lambda: projection(1)}[ph]()

    nc.compile()
    return nc


def _fold_weights(w, out_dim):
    """w: (330, out). Returns [5, 66, out] with the reference's x0c-mutation
    linear combinations folded in and rows reordered hx-first."""
    Wm = w.reshape(66, 5, out_dim)
    What = np.stack(
        [
            Wm[:, 0] - Wm[:, 2],
            Wm[:, 1] - Wm[:, 4],
            2.0 * Wm[:, 2],
            Wm[:, 3],
            2.0 * Wm[:, 4],
        ]
    )  # [5, 66, out]
    What = np.concatenate([What[:, 2:, :], What[:, :2, :]], axis=1)  # hx rows first
    return What.astype(np.float32)


def _q8(x, scale):
    return np.clip(x.astype(np.float32) * scale, -240.0, 240.0).astype(E4)


_NC_CACHE = {}


def _get_nc(N):
    if N not in _NC_CACHE:
        _NC_CACHE[N] = _build_nc(N)
    return _NC_CACHE[N]


def _prepare_in_maps(inputs, hx, supports, w_fn, b_fn, w_g, b_g):
    inputs = np.ascontiguousarray(np.asarray(inputs), dtype=np.float32)
    hx = np.ascontiguousarray(np.asarray(hx), dtype=np.float32)
    supports = np.ascontiguousarray(np.asarray(supports), dtype=np.float32)
    w_fn = np.asarray(w_fn, dtype=np.float32)
    b_fn = np.asarray(b_fn, dtype=np.float32)
    w_g = np.asarray(w_g, dtype=np.float32)
    b_g = np.asarray(b_g, dtype=np.float32)

    N = supports.shape[1]
    NB = N // 128

    # ---- replicated tensors ----
    # stb[s, nb, kp, kb*128+m] = supports[s][nb*128+m, kb*128+kp], fp8 x 2^17
    stb = _q8(
        np.ascontiguousarray(
            supports.reshape(2, NB, 128, NB, 128).transpose(0, 1, 4, 3, 2)
        ).reshape(2, NB, 128, NB * 128),
        SSC,
    )
    WhatF = _fold_weights(w_fn, 128)  # [5, 66, 128]
    WhatG = _fold_weights(w_g, 64)
    # all projection terms tau-scaled in PSUM (activation descales)
    wfn_h = np.ascontiguousarray(WhatF[0] * TAU).astype(ml_dtypes.bfloat16)
    wg_h = np.ascontiguousarray(WhatG[0] * TAU).astype(ml_dtypes.bfloat16)
    # m>=1 weights: fp8, tau-scaled and descaled by the hop state scale
    wfn8_h = _q8(
        np.ascontiguousarray((WhatF[1:] / AH).transpose(1, 0, 2).reshape(66, 4 * 128)),
        TAU,
    )
    wg8_h = _q8(
        np.ascontiguousarray((WhatG[1:] / AH).transpose(1, 0, 2).reshape(66, 4 * 64)),
        TAU,
    )
    bfn_h = b_fn.reshape(128, 1).copy()
    bg_h = b_g.reshape(64, 1).copy()

    in_maps = []
    for c in range(NCORES):
        sl = slice(c * BLOC, (c + 1) * BLOC)
        inp_c = inputs[sl].reshape(BLOC, N, IN_DIM)
        hx_c = hx[sl].reshape(BLOC, N, UNITS)
        # X0 [N, 528]: hx cols b*64+u, input cols 512 + b*2 + j
        x0 = np.concatenate(
            [
                hx_c.transpose(1, 0, 2).reshape(N, CHX),
                inp_c.transpose(1, 0, 2).reshape(N, CIN),
            ],
            axis=1,
        )
        x0pm = _q8(
            np.ascontiguousarray(x0.reshape(NB, 128, C).transpose(1, 0, 2)).reshape(
                128, NB * C
            ),
            A0,
        )
        xin = x0[:, CHX:]
        xint = np.ascontiguousarray(xin.T).astype(ml_dtypes.bfloat16)
        hxt = np.ascontiguousarray(hx_c.transpose(0, 2, 1)).astype(ml_dtypes.bfloat16)
        in_maps.append(
            {
                "x0pm": x0pm,
                "stb": stb,
                "xint": xint,
                "hxt": hxt,
                "wfn": wfn_h,
                "wg": wg_h,
                "wfn8": wfn8_h,
                "wg8": wg8_h,
                "bfn": bfn_h,
                "bg": bg_h,
            }
        )

    return in_maps


def kernel(inputs, hx, supports, w_fn, b_fn, w_g, b_g):
    N = np.asarray(supports).shape[1]
    nc = _get_nc(N)
    in_maps = _prepare_in_maps(inputs, hx, supports, w_fn, b_fn, w_g, b_g)
    kernel.last_in_maps = in_maps
    res = run_bass_kernel_spmd(
        nc,
        in_maps,
        core_ids=list(range(NCORES)),
        trace=bool(int(os.environ.get("DCGRU_TRACE", "0"))),
    )

    out = np.empty((B, N * UNITS), np.float32)
    for c in range(NCORES):
        outt = res.results[c]["outt"]  # [BLOC, UNITS, N]
        out[c * BLOC : (c + 1) * BLOC] = outt.transpose(0, 2, 1).reshape(BLOC, -1)
    kernel.last_results = res
    return out
